# revision 2
# baseline (speedup 1.0000x reference)
"""Per-pixel dynamic 5x5 conv (KernelConv2d) + leaky-relu, data-parallel on 8 TRN2 cores.

Sharding: core i <- (n = i//2, h-half = i%2); each core computes out[n, :, h0:h0+128, :].

v9 design (fp8e3 kernel stream, (w,c)-interleaved layout, 3-route tap fan-out):
- x rows stored [wpad, c]-interleaved: every dx window is a contiguous,
  16B-aligned 2048-elem read -> all DVE products run in 2x_1P at ~1135ns,
  no odd-alignment copy needed.
- kernel tensor quantized host-side to TRN fp8 E3M4 (4 mantissa bits;
  max|k|=5.4 < 15.5; measured end-to-end rel err 0.0155 < 2e-2) halving its
  HBM footprint. Taps reach fp16 product planes via 3 routes chosen to
  balance the SBUF AXI write fabric, DVE, Pool and ACT:
    CAST taps (16): SWDGE cast-DMA fp8->fp16 (gpsimd ring), DVE multiplies.
    ACT taps  (5): loaded raw fp8 (HWDGE), ScalarE upconverts (engine port,
                   saves AXI-write bytes), DVE multiplies.
    POOL taps (4): loaded raw fp8 (HWDGE), GpSimd multiplies fp16 x fp8
                   directly into dedicated buffers.
- PE accumulates all 26 planes as shifted-identity matmuls in fp32 PSUM
  (ldweights reused within each dy run); shift matrices ride a small DRAM
  const DMA instead of being built on-chip.
- Tail: per 512-col chunk, stop-matmul -> ACT copies PSUM->SBUF fp16 ->
  DVE scalar_tensor_tensor max(0.2*x, x) -> output DMA on the sync ring
  (input rides scalar + gpsimd rings, so sync's ring is free at the end).
- Bottom-boundary rows (x rows 128..131) via the 50-partition tail product
  + scatter matmul at queue position 13, as in v8.
"""

import os
from contextlib import ExitStack

import numpy as np
import ml_dtypes

import concourse.bass as bass
import concourse.mybir as mybir
from concourse.bass_utils import run_bass_kernel_spmd

N, C, H, W = 4, 8, 256, 256
K = 5
PAD = 2
NCORES = 8
HSH = H // 2              # 128 output rows per core
XW = (W + 4) * C          # 2080 stored x row width ((w,c) interleaved)
CD = mybir.dt.float16
F8 = mybir.dt.float8e3
NEG = 0.2
NB = 8                    # DVE product ring buffers
NWARM = 8                 # PE warm-up dummy matmuls (HAM clock-gate)
NTAP = K * K              # 25
COMBOS = [(dy, p) for dy in (1, 2, 3, 4) for p in range(HSH - dy, HSH)]
NTAIL = len(COMBOS) * K   # 50
TAILPOS = 13              # queue position of the tail product
CW = C * W                # 2048
FD = CW                   # product plane free dim

POOL_TAPS = [17, 19, 21, 23]
ACT_TAPS = [8, 10, 12, 14, 16]
CAST_TAPS = [p for p in range(NTAP) if p not in POOL_TAPS and p not in ACT_TAPS]
# cast DMA groups: two singletons for a fast head, then pairs
CAST_DMAS = [[CAST_TAPS[0]], [CAST_TAPS[1]]] + [
    CAST_TAPS[i:i + 2] for i in range(2, len(CAST_TAPS), 2)]
NKC = len(CAST_DMAS)
KTSLOT = {p: i for i, p in enumerate(CAST_TAPS)}
for i, p in enumerate(ACT_TAPS):
    KTSLOT[p] = len(CAST_TAPS) + i
K8SLOT = {p: i for i, p in enumerate(ACT_TAPS + POOL_TAPS)}
NKT = len(CAST_TAPS) + len(ACT_TAPS)   # 21 fp16 tap slots
NK8 = len(ACT_TAPS) + len(POOL_TAPS)   # 9 raw fp8 tap slots

_NC_CACHE = {}


def _build_nc():
    nc = bass.Bass("TRN2", target_bir_lowering=False, debug=False,
                   num_devices=NCORES)
    xe_d = nc.dram_tensor("xe", [HSH, XW], CD, kind="ExternalInput").ap()
    wt_d = nc.dram_tensor("wt", [HSH, K, HSH], CD, kind="ExternalInput").ap()
    k8_d = nc.dram_tensor("k8", [HSH, NK8, FD], F8, kind="ExternalInput").ap()
    kc_d = nc.dram_tensor("kc", [HSH, len(CAST_TAPS), FD], F8,
                          kind="ExternalInput").ap()
    tl_d = nc.dram_tensor("tl", [NTAIL, 2 * CW + HSH], CD,
                          kind="ExternalInput").ap()
    out_d = nc.dram_tensor("out", [HSH, FD], CD, kind="ExternalOutput").ap()

    with ExitStack() as ctx:
        xe = ctx.enter_context(nc.sbuf_tensor("xe_s", [HSH, XW], CD))
        wt = ctx.enter_context(nc.sbuf_tensor("wt_s", [HSH, K, HSH], CD))
        kt = ctx.enter_context(nc.sbuf_tensor("kt_s", [HSH, NKT, FD], CD))
        k8 = ctx.enter_context(nc.sbuf_tensor("k8_s", [HSH, NK8, FD], F8))
        tl = ctx.enter_context(nc.sbuf_tensor("tl_s", [NTAIL, 2 * CW + HSH], CD))
        prod = [ctx.enter_context(nc.sbuf_tensor(f"pr{b}", [HSH, FD], CD))
                for b in range(NB)]
        pprod = [ctx.enter_context(nc.sbuf_tensor(f"pp{b}", [HSH, FD], CD))
                 for b in range(len(POOL_TAPS))]
        ptail = ctx.enter_context(nc.sbuf_tensor("ptail", [NTAIL, FD], CD))
        tmpx = ctx.enter_context(nc.sbuf_tensor("tmpx", [HSH, FD], CD))
        ot = ctx.enter_context(nc.sbuf_tensor("ot", [HSH, FD], CD))
        pt = ctx.enter_context(nc.psum_tensor("pt", [HSH, FD], mybir.dt.float32))
        scr = ctx.enter_context(nc.psum_tensor("scr", [HSH, 512], mybir.dt.float32))

        xt = tl[:, 0:CW]
        ktl = tl[:, CW:2 * CW]
        wtl = tl[:, 2 * CW:]                      # [50, 128] scatter matrix

        s_xe = ctx.enter_context(nc.semaphore("s_xe"))
        s_w = ctx.enter_context(nc.semaphore("s_w"))
        s_k8 = ctx.enter_context(nc.semaphore("s_k8"))
        s_tl = ctx.enter_context(nc.semaphore("s_tl"))
        s_kc = [ctx.enter_context(nc.semaphore(f"s_kc{j}")) for j in range(NKC)]
        s_ac = ctx.enter_context(nc.semaphore("s_ac"))  # ACT tap converts
        s_v = ctx.enter_context(nc.semaphore("s_v"))    # DVE queue items
        s_pv = ctx.enter_context(nc.semaphore("s_pv"))  # Pool products
        s_mm = ctx.enter_context(nc.semaphore("s_mm"))  # ring items consumed
        s_c = ctx.enter_context(nc.semaphore("s_c"))    # per-chunk stop MMs
        s_t = ctx.enter_context(nc.semaphore("s_t"))    # ACT psum->sbuf chunks
        s_e = ctx.enter_context(nc.semaphore("s_e"))    # lrelu chunks done
        s_o = ctx.enter_context(nc.semaphore("s_o"))
        block = ctx.enter_context(nc.Block())

        # queue: 25 taps in (dy, dx) order with the tail inserted at TAILPOS
        queue = []
        for p in range(NTAP):
            if len(queue) == TAILPOS:
                queue.append(('L', -1))
            queue.append(('T', p))
        # DVE-produced items in queue order (taps not on Pool, plus tail)
        dve_items = [it for it in queue if it[0] == 'L' or it[1] not in POOL_TAPS]
        sv_count = {}   # queue index -> s_v threshold when this item is ready
        n = 0
        for qi, it in enumerate(queue):
            if it[0] == 'L' or it[1] not in POOL_TAPS:
                n += 1
                sv_count[qi] = n
        ring_items = [it for it in dve_items if it[0] == 'T']  # ring-buffered
        ring_idx = {}   # tap -> ring index
        for i, it in enumerate(ring_items):
            ring_idx[it[1]] = i

        @block.sync
        def _(sync):
            sync.dma_start(xe[:], xe_d).then_inc(s_xe, 16)
            for q in range(4):
                sync.wait_ge(s_e, q + 1)
                sync.dma_start(out_d[:, 512 * q:512 * (q + 1)],
                               ot[:, 512 * q:512 * (q + 1)]).then_inc(s_o, 16)
            sync.wait_ge(s_o, 64)

        @block.scalar
        def _(scalar):
            scalar.dma_start(wt[:], wt_d).then_inc(s_w, 16)
            scalar.dma_start(k8[:], k8_d).then_inc(s_k8, 16)
            scalar.dma_start(tl[:], tl_d).then_inc(s_tl, 16)
            scalar.wait_ge(s_k8, 16)
            for i, p in enumerate(ACT_TAPS):
                scalar.activation(kt[:, KTSLOT[p]], k8[:, K8SLOT[p]],
                                  mybir.ActivationFunctionType.Copy,
                                  bias=0.0, scale=1.0).then_inc(s_ac, 1)
            for q in range(4):
                scalar.wait_ge(s_c, q + 1)
                scalar.activation(tmpx[:, 512 * q:512 * (q + 1)],
                                  pt[:, 512 * q:512 * (q + 1)],
                                  mybir.ActivationFunctionType.Copy,
                                  bias=0.0, scale=1.0).then_inc(s_t, 1)

        @block.gpsimd
        def _(gpsimd):
            for j, taps in enumerate(CAST_DMAS):
                t0 = KTSLOT[taps[0]]
                gpsimd.dma_start(kt[:, t0:t0 + len(taps)],
                                 kc_d[:, t0:t0 + len(taps)]).then_inc(s_kc[j], 16)
            gpsimd.wait_ge(s_k8, 16)
            gpsimd.wait_ge(s_xe, 16)
            for i, p in enumerate(POOL_TAPS):
                dx = p % K
                gpsimd.tensor_tensor(pprod[i][:],
                                     xe[:, C * dx:C * dx + FD],
                                     k8[:, K8SLOT[p]],
                                     op=mybir.AluOpType.mult).then_inc(s_pv, 1)

        @block.vector
        def _(vector):
            vector.wait_ge(s_xe, 16)
            act_seen = 0
            kc_seen = 0
            for qi, it in enumerate(queue):
                if it[0] == 'L':
                    vector.wait_ge(s_tl, 16)
                    vector.tensor_tensor(ptail[0:NTAIL], xt[0:NTAIL],
                                         ktl[0:NTAIL],
                                         op=mybir.AluOpType.mult).then_inc(s_v, 1)
                    continue
                p = it[1]
                if p in POOL_TAPS:
                    continue
                slot = KTSLOT[p]
                if p in ACT_TAPS:
                    act_seen = ACT_TAPS.index(p) + 1
                    vector.wait_ge(s_ac, act_seen)
                else:
                    j = next(j for j, taps in enumerate(CAST_DMAS) if p in taps)
                    if j >= kc_seen:
                        kc_seen = j + 1
                        vector.wait_ge(s_kc[j], 16)
                r = ring_idx[p]
                if r >= NB and r % 4 == 0:
                    vector.wait_ge(s_mm, r - 4)
                dx = p % K
                vector.tensor_tensor(prod[r % NB][:],
                                     xe[:, C * dx:C * dx + FD],
                                     kt[:, slot],
                                     op=mybir.AluOpType.mult).then_inc(s_v, 1)
            for q in range(4):
                vector.wait_ge(s_t, q + 1)
                vector.scalar_tensor_tensor(
                    ot[:, 512 * q:512 * (q + 1)],
                    tmpx[:, 512 * q:512 * (q + 1)], NEG,
                    tmpx[:, 512 * q:512 * (q + 1)],
                    op0=mybir.AluOpType.mult,
                    op1=mybir.AluOpType.max).then_inc(s_e, 1)

        @block.tensor
        def _(tensor):
            for r in range(NWARM):
                mm = tensor.matmul(scr[:], lhsT=ot[:, 0:HSH],
                                   rhs=ot[:, 0:512], start=True, stop=True)
                if r > 0:
                    mm.ins.ldweights = False
            tensor.wait_ge(s_w, 16)
            prev_w = [-1]

            def mmul(rhs_ap, q, wid, start, stop):
                mm = tensor.matmul(pt[:, 512 * q:512 * (q + 1)],
                                   lhsT=(wtl[0:NTAIL] if wid == 5
                                         else wt[:, wid]),
                                   rhs=rhs_ap,
                                   start=start, stop=stop)
                if wid == prev_w[0]:
                    mm.ins.ldweights = False
                prev_w[0] = wid
                return mm

            pool_seen = 0
            for qi, it in enumerate(queue):
                first, last = qi == 0, qi == len(queue) - 1
                if it[0] == 'L':
                    tensor.wait_ge(s_v, sv_count[qi])
                    for q in range(4):
                        mm = mmul(ptail[0:NTAIL, 512 * q:512 * (q + 1)], q, 5,
                                  first, last)
                    continue
                p = it[1]
                dy = p // K
                if p in POOL_TAPS:
                    pool_seen = POOL_TAPS.index(p) + 1
                    tensor.wait_ge(s_pv, pool_seen)
                    src = pprod[POOL_TAPS.index(p)]
                else:
                    tensor.wait_ge(s_v, sv_count[qi])
                    src = prod[ring_idx[p] % NB]
                for q in range(4):
                    mm = mmul(src[:, 512 * q:512 * (q + 1)], q, dy,
                              first, last)
                    if last:
                        mm.then_inc(s_c, 1)
                if not last and p not in POOL_TAPS:
                    mm.then_inc(s_mm, 1)
    return nc


def get_nc():
    if "nc" not in _NC_CACHE:
        _NC_CACHE["nc"] = _build_nc()
    return _NC_CACHE["nc"]


def _prep_shards(x: np.ndarray, kernel: np.ndarray):
    """Host-side: pad, quantize kernel to e3m4, build (w,c)-interleaved shards."""
    f16 = np.float16
    f8 = ml_dtypes.float8_e3m4
    xp = np.pad(x, ((0, 0), (0, 0), (PAD, PAD), (PAD, PAD)),
                mode='edge').astype(f16)              # (N, C, 260, 260)
    k8full = kernel.astype(f8)                         # quantize once
    kr8 = k8full.reshape(N, C, NTAP, H, W)
    kr16 = kernel.astype(f16).reshape(N, C, NTAP, H, W)

    # shift-matrix blob (shared by all cores)
    wtb = np.zeros((HSH, K, HSH), f16)
    for dy in range(K):
        for q in range(dy, HSH):
            wtb[q, dy, q - dy] = 1.0

    in_maps = []
    for core in range(NCORES):
        n, hb = divmod(core, 2)
        h0 = hb * HSH
        blk = xp[n, :, h0:h0 + HSH + 4, :]             # (C, 132, 260)
        # x rows, (w, c) interleaved
        xeb = np.ascontiguousarray(
            blk[:, :HSH, :].transpose(1, 2, 0)).reshape(HSH, XW)

        kb8 = kr8[n, :, :, h0:h0 + HSH, :]             # (C, 25, 128, W) e3m4
        kcb = np.zeros((HSH, len(CAST_TAPS), FD), f8)
        k8b = np.zeros((HSH, NK8, FD), f8)
        for p in CAST_TAPS:
            dy = p // K
            kcb[dy:, KTSLOT[p]] = kb8[:, p, :HSH - dy].transpose(
                1, 2, 0).reshape(HSH - dy, FD)
        for p in ACT_TAPS + POOL_TAPS:
            dy = p // K
            k8b[dy:, K8SLOT[p]] = kb8[:, p, :HSH - dy].transpose(
                1, 2, 0).reshape(HSH - dy, FD)

        kb16 = kr16[n, :, :, h0:h0 + HSH, :]           # (C, 25, 128, W) fp16
        tlb = np.zeros((NTAIL, 2 * CW + HSH), f16)
        for j, (dy, p) in enumerate(COMBOS):
            for dx in range(K):
                tlb[j * K + dx, 0:CW] = blk[:, p + dy, dx:dx + W].T.reshape(CW)
                tlb[j * K + dx, CW:2 * CW] = kb16[:, dy * K + dx, p].T.reshape(CW)
                tlb[j * K + dx, 2 * CW + p] = 1.0

        in_maps.append({"xe": xeb, "wt": wtb, "k8": k8b, "kc": kcb, "tl": tlb})
    return in_maps


def kernel(x: np.ndarray, kernel: np.ndarray) -> np.ndarray:
    nc = get_nc()
    in_maps = _prep_shards(np.asarray(x), np.asarray(kernel))
    trace = bool(int(os.environ.get("KC_TRACE", "0")))
    res = run_bass_kernel_spmd(nc, in_maps, core_ids=list(range(NCORES)),
                               trace=trace)
    _NC_CACHE["last_results"] = res
    out = np.empty((N, C, H, W), np.float32)
    for core in range(NCORES):
        n, hb = divmod(core, 2)
        h0 = hb * HSH
        o = res.results[core]["out"]  # (128, 2048) fp16, (w, c) interleaved
        out[n, :, h0:h0 + HSH, :] = o.reshape(HSH, W, C).transpose(
            2, 0, 1).astype(np.float32)
    return out


# revision 10
# speedup vs baseline: 1.0874x; 1.0874x over previous
"""Per-pixel dynamic 5x5 conv (KernelConv2d) + leaky-relu, data-parallel on 8 TRN2 cores.

Sharding: core i <- (n = i//2, h-half = i%2); each core computes out[n, :, h0:h0+128, :].

v9 design (fp8e3 kernel stream, (w,c)-interleaved layout, 3-route tap fan-out):
- x rows stored [wpad, c]-interleaved: every dx window is a contiguous,
  16B-aligned 2048-elem read -> all DVE products run in 2x_1P at ~1135ns,
  no odd-alignment copy needed.
- kernel tensor quantized host-side to TRN fp8 E3M4 (4 mantissa bits;
  max|k|=5.4 < 15.5; measured end-to-end rel err 0.0155 < 2e-2) halving its
  HBM footprint. Taps reach fp16 product planes via 3 routes chosen to
  balance the SBUF AXI write fabric, DVE, Pool and ACT:
    CAST taps (16): SWDGE cast-DMA fp8->fp16 (gpsimd ring), DVE multiplies.
    ACT taps  (9): loaded raw fp8 (HWDGE, issue gated behind the first cast
                   DMAs), ScalarE upconverts via its engine port (saves
                   AXI-write fabric bytes), DVE multiplies.
    (GpSimd tensor ops are NOT used for products: Pool TT contends with DVE
    TT on the shared SBUF port pair and slows both ~4x.)
- PE accumulates all 26 planes as shifted-identity matmuls in fp32 PSUM
  (ldweights reused within each dy run); shift matrices ride a small DRAM
  const DMA instead of being built on-chip.
- Tail: per 512-col chunk, stop-matmul -> ACT copies PSUM->SBUF fp16 ->
  DVE scalar_tensor_tensor max(0.2*x, x) -> output DMA on the sync ring
  (input rides scalar + gpsimd rings, so sync's ring is free at the end).
- Bottom-boundary rows (x rows 128..131) via the 50-partition tail product
  + scatter matmul at queue position 13, as in v8.
"""

import os
from contextlib import ExitStack

import numpy as np
import ml_dtypes

import concourse.bass as bass
import concourse.mybir as mybir
from concourse.bass_utils import run_bass_kernel_spmd

N, C, H, W = 4, 8, 256, 256
K = 5
PAD = 2
NCORES = 8
HSH = H // 2              # 128 output rows per core
XW = (W + 4) * C          # 2080 stored x row width ((w,c) interleaved)
CD = mybir.dt.float16
F8 = mybir.dt.float8e3
NEG = 0.2
NB = 8                    # DVE product ring buffers
NWARM = 8                 # PE warm-up dummy matmuls (HAM clock-gate)
NTAP = K * K              # 25
COMBOS = [(dy, p) for dy in (1, 2, 3, 4) for p in range(HSH - dy, HSH)]
NTAIL = len(COMBOS) * K   # 50
TAILPOS = 13              # queue position of the tail product
CW = C * W                # 2048
FD = CW                   # product plane free dim

POOL_TAPS = []                         # Pool TT contends with DVE TT: unused
ACT_TAPS = [8, 10, 12, 14, 16, 18, 20, 22, 24]
ACT_DMAS = [ACT_TAPS[0:3], ACT_TAPS[3:]]   # k8 arrives in two pieces
CAST_TAPS = [p for p in range(NTAP) if p not in POOL_TAPS and p not in ACT_TAPS]
# cast DMA groups: three singletons for a fast head, then pairs
CAST_DMAS = [[CAST_TAPS[0]], [CAST_TAPS[1]], [CAST_TAPS[2]]] + [
    CAST_TAPS[i:i + 2] for i in range(3, len(CAST_TAPS), 2)]
NKC = len(CAST_DMAS)
KTSLOT = {p: i for i, p in enumerate(CAST_TAPS)}
for i, p in enumerate(ACT_TAPS):
    KTSLOT[p] = len(CAST_TAPS) + i
K8SLOT = {p: i for i, p in enumerate(ACT_TAPS + POOL_TAPS)}
NKT = len(CAST_TAPS) + len(ACT_TAPS)   # 25 fp16 tap slots
NK8 = len(ACT_TAPS) + len(POOL_TAPS)   # 9 raw fp8 tap slots

_NC_CACHE = {}


def _build_nc():
    nc = bass.Bass("TRN2", target_bir_lowering=False, debug=False,
                   num_devices=NCORES)
    xe_d = nc.dram_tensor("xe", [HSH, XW], CD, kind="ExternalInput").ap()
    wt_d = nc.dram_tensor("wt", [HSH, K, HSH], CD, kind="ExternalInput").ap()
    k8_d = nc.dram_tensor("k8", [HSH, NK8, FD], F8, kind="ExternalInput").ap()
    kc_d = nc.dram_tensor("kc", [HSH, len(CAST_TAPS), FD], F8,
                          kind="ExternalInput").ap()
    tl_d = nc.dram_tensor("tl", [NTAIL, 2 * CW + HSH], CD,
                          kind="ExternalInput").ap()
    out_d = nc.dram_tensor("out", [HSH, FD], CD, kind="ExternalOutput").ap()

    with ExitStack() as ctx:
        xe = ctx.enter_context(nc.sbuf_tensor("xe_s", [HSH, XW], CD))
        wt = ctx.enter_context(nc.sbuf_tensor("wt_s", [HSH, K, HSH], CD))
        kt = ctx.enter_context(nc.sbuf_tensor("kt_s", [HSH, NKT, FD], CD))
        k8 = ctx.enter_context(nc.sbuf_tensor("k8_s", [HSH, NK8, FD], F8))
        tl = ctx.enter_context(nc.sbuf_tensor("tl_s", [NTAIL, 2 * CW + HSH], CD))
        prod = [ctx.enter_context(nc.sbuf_tensor(f"pr{b}", [HSH, FD], CD))
                for b in range(NB)]
        ptail = ctx.enter_context(nc.sbuf_tensor("ptail", [NTAIL, FD], CD))
        tmpx = ctx.enter_context(nc.sbuf_tensor("tmpx", [HSH, FD], CD))
        ot = ctx.enter_context(nc.sbuf_tensor("ot", [HSH, FD], CD))
        pt = ctx.enter_context(nc.psum_tensor("pt", [HSH, FD], mybir.dt.float32))
        scr = ctx.enter_context(nc.psum_tensor("scr", [HSH, 512], mybir.dt.float32))

        xt = tl[:, 0:CW]
        ktl = tl[:, CW:2 * CW]
        wtl = tl[:, 2 * CW:]                      # [50, 128] scatter matrix

        s_xe = ctx.enter_context(nc.semaphore("s_xe"))
        s_w = ctx.enter_context(nc.semaphore("s_w"))
        s_k8a = ctx.enter_context(nc.semaphore("s_k8a"))
        s_k8b = ctx.enter_context(nc.semaphore("s_k8b"))
        s_tl = ctx.enter_context(nc.semaphore("s_tl"))
        s_kc = [ctx.enter_context(nc.semaphore(f"s_kc{j}")) for j in range(NKC)]
        s_ac = ctx.enter_context(nc.semaphore("s_ac"))  # ACT tap converts
        s_v = ctx.enter_context(nc.semaphore("s_v"))    # DVE queue items
        s_mm = ctx.enter_context(nc.semaphore("s_mm"))  # ring items consumed
        s_c = ctx.enter_context(nc.semaphore("s_c"))    # per-chunk stop MMs
        s_t = ctx.enter_context(nc.semaphore("s_t"))    # ACT psum->sbuf chunks
        s_e = ctx.enter_context(nc.semaphore("s_e"))    # lrelu chunks done
        s_o = ctx.enter_context(nc.semaphore("s_o"))
        block = ctx.enter_context(nc.Block())

        # queue: 25 taps in (dy, dx) order with the tail inserted at TAILPOS
        queue = []
        for p in range(NTAP):
            if len(queue) == TAILPOS:
                queue.append(('L', -1))
            queue.append(('T', p))
        # DVE-produced items in queue order (taps not on Pool, plus tail)
        dve_items = [it for it in queue if it[0] == 'L' or it[1] not in POOL_TAPS]
        sv_count = {}   # queue index -> s_v threshold when this item is ready
        n = 0
        for qi, it in enumerate(queue):
            if it[0] == 'L' or it[1] not in POOL_TAPS:
                n += 1
                sv_count[qi] = n
        ring_items = [it for it in dve_items if it[0] == 'T']  # ring-buffered
        ring_idx = {}   # tap -> ring index
        for i, it in enumerate(ring_items):
            ring_idx[it[1]] = i

        @block.sync
        def _(sync):
            sync.dma_start(xe[:], xe_d).then_inc(s_xe, 16)
            for q in range(4):
                sync.wait_ge(s_e, q + 1)
                sync.dma_start(out_d[:, 512 * q:512 * (q + 1)],
                               ot[:, 512 * q:512 * (q + 1)]).then_inc(s_o, 16)
            sync.wait_ge(s_o, 64)

        @block.scalar
        def _(scalar):
            scalar.dma_start(wt[:], wt_d).then_inc(s_w, 16)
            # let the cast ring own the early fabric: gate k8/tl issue
            scalar.wait_ge(s_kc[2], 16)
            na = len(ACT_DMAS[0])
            scalar.dma_start(k8[:, 0:na], k8_d[:, 0:na]).then_inc(s_k8a, 16)
            scalar.dma_start(k8[:, na:], k8_d[:, na:]).then_inc(s_k8b, 16)
            scalar.dma_start(tl[:], tl_d).then_inc(s_tl, 16)
            for i, p in enumerate(ACT_TAPS):
                scalar.wait_ge(s_k8a if i < na else s_k8b, 16)
                scalar.activation(kt[:, KTSLOT[p]], k8[:, K8SLOT[p]],
                                  mybir.ActivationFunctionType.Copy,
                                  bias=0.0, scale=1.0).then_inc(s_ac, 1)
            for q in range(4):
                scalar.wait_ge(s_c, q + 1)
                scalar.activation(tmpx[:, 512 * q:512 * (q + 1)],
                                  pt[:, 512 * q:512 * (q + 1)],
                                  mybir.ActivationFunctionType.Copy,
                                  bias=0.0, scale=1.0).then_inc(s_t, 1)

        @block.gpsimd
        def _(gpsimd):
            for j, taps in enumerate(CAST_DMAS):
                t0 = KTSLOT[taps[0]]
                gpsimd.dma_start(kt[:, t0:t0 + len(taps)],
                                 kc_d[:, t0:t0 + len(taps)]).then_inc(s_kc[j], 16)

        @block.vector
        def _(vector):
            vector.wait_ge(s_xe, 16)
            act_seen = 0
            kc_seen = 0
            for qi, it in enumerate(queue):
                if it[0] == 'L':
                    vector.wait_ge(s_tl, 16)
                    vector.tensor_tensor(ptail[0:NTAIL], xt[0:NTAIL],
                                         ktl[0:NTAIL],
                                         op=mybir.AluOpType.mult).then_inc(s_v, 1)
                    continue
                p = it[1]
                if p in POOL_TAPS:
                    continue
                slot = KTSLOT[p]
                if p in ACT_TAPS:
                    act_seen = ACT_TAPS.index(p) + 1
                    vector.wait_ge(s_ac, act_seen)
                else:
                    j = next(j for j, taps in enumerate(CAST_DMAS) if p in taps)
                    if j >= kc_seen:
                        kc_seen = j + 1
                        vector.wait_ge(s_kc[j], 16)
                r = ring_idx[p]
                if r >= NB and r % 4 == 0:
                    vector.wait_ge(s_mm, r - 4)
                dx = p % K
                vector.tensor_tensor(prod[r % NB][:],
                                     xe[:, C * dx:C * dx + FD],
                                     kt[:, slot],
                                     op=mybir.AluOpType.mult).then_inc(s_v, 1)
            for q in range(4):
                vector.wait_ge(s_t, q + 1)
                vector.scalar_tensor_tensor(
                    ot[:, 512 * q:512 * (q + 1)],
                    tmpx[:, 512 * q:512 * (q + 1)], NEG,
                    tmpx[:, 512 * q:512 * (q + 1)],
                    op0=mybir.AluOpType.mult,
                    op1=mybir.AluOpType.max).then_inc(s_e, 1)

        @block.tensor
        def _(tensor):
            for r in range(NWARM):
                mm = tensor.matmul(scr[:], lhsT=ot[:, 0:HSH],
                                   rhs=ot[:, 0:512], start=True, stop=True)
                if r > 0:
                    mm.ins.ldweights = False
            tensor.wait_ge(s_w, 16)
            prev_w = [-1]

            def mmul(rhs_ap, q, wid, start, stop):
                mm = tensor.matmul(pt[:, 512 * q:512 * (q + 1)],
                                   lhsT=(wtl[0:NTAIL] if wid == 5
                                         else wt[:, wid]),
                                   rhs=rhs_ap,
                                   start=start, stop=stop)
                if wid == prev_w[0]:
                    mm.ins.ldweights = False
                prev_w[0] = wid
                return mm

            for qi, it in enumerate(queue):
                first, last = qi == 0, qi == len(queue) - 1
                if it[0] == 'L':
                    tensor.wait_ge(s_v, sv_count[qi])
                    for q in range(4):
                        mm = mmul(ptail[0:NTAIL, 512 * q:512 * (q + 1)], q, 5,
                                  first, last)
                    continue
                p = it[1]
                dy = p // K
                tensor.wait_ge(s_v, sv_count[qi])
                src = prod[ring_idx[p] % NB]
                for q in range(4):
                    mm = mmul(src[:, 512 * q:512 * (q + 1)], q, dy,
                              first, last)
                    if last:
                        mm.then_inc(s_c, 1)
                if not last:
                    mm.then_inc(s_mm, 1)
    return nc


def get_nc():
    if "nc" not in _NC_CACHE:
        _NC_CACHE["nc"] = _build_nc()
    return _NC_CACHE["nc"]


def _prep_shards(x: np.ndarray, kernel: np.ndarray):
    """Host-side: pad, quantize kernel to e3m4, build (w,c)-interleaved shards."""
    f16 = np.float16
    f8 = ml_dtypes.float8_e3m4
    xp = np.pad(x, ((0, 0), (0, 0), (PAD, PAD), (PAD, PAD)),
                mode='edge').astype(f16)              # (N, C, 260, 260)
    k8full = kernel.astype(f8)                         # quantize once
    kr8 = k8full.reshape(N, C, NTAP, H, W)
    kr16 = kernel.astype(f16).reshape(N, C, NTAP, H, W)

    # shift-matrix blob (shared by all cores)
    wtb = np.zeros((HSH, K, HSH), f16)
    for dy in range(K):
        for q in range(dy, HSH):
            wtb[q, dy, q - dy] = 1.0

    in_maps = []
    for core in range(NCORES):
        n, hb = divmod(core, 2)
        h0 = hb * HSH
        blk = xp[n, :, h0:h0 + HSH + 4, :]             # (C, 132, 260)
        # x rows, (w, c) interleaved
        xeb = np.ascontiguousarray(
            blk[:, :HSH, :].transpose(1, 2, 0)).reshape(HSH, XW)

        kb8 = kr8[n, :, :, h0:h0 + HSH, :]             # (C, 25, 128, W) e3m4
        kcb = np.zeros((HSH, len(CAST_TAPS), FD), f8)
        k8b = np.zeros((HSH, NK8, FD), f8)
        for p in CAST_TAPS:
            dy = p // K
            kcb[dy:, KTSLOT[p]] = kb8[:, p, :HSH - dy].transpose(
                1, 2, 0).reshape(HSH - dy, FD)
        for p in ACT_TAPS + POOL_TAPS:
            dy = p // K
            k8b[dy:, K8SLOT[p]] = kb8[:, p, :HSH - dy].transpose(
                1, 2, 0).reshape(HSH - dy, FD)

        kb16 = kr16[n, :, :, h0:h0 + HSH, :]           # (C, 25, 128, W) fp16
        tlb = np.zeros((NTAIL, 2 * CW + HSH), f16)
        for j, (dy, p) in enumerate(COMBOS):
            for dx in range(K):
                tlb[j * K + dx, 0:CW] = blk[:, p + dy, dx:dx + W].T.reshape(CW)
                tlb[j * K + dx, CW:2 * CW] = kb16[:, dy * K + dx, p].T.reshape(CW)
                tlb[j * K + dx, 2 * CW + p] = 1.0

        in_maps.append({"xe": xeb, "wt": wtb, "k8": k8b, "kc": kcb, "tl": tlb})
    return in_maps


def kernel(x: np.ndarray, kernel: np.ndarray) -> np.ndarray:
    nc = get_nc()
    in_maps = _prep_shards(np.asarray(x), np.asarray(kernel))
    trace = bool(int(os.environ.get("KC_TRACE", "0")))
    res = run_bass_kernel_spmd(nc, in_maps, core_ids=list(range(NCORES)),
                               trace=trace)
    _NC_CACHE["last_results"] = res
    out = np.empty((N, C, H, W), np.float32)
    for core in range(NCORES):
        n, hb = divmod(core, 2)
        h0 = hb * HSH
        o = res.results[core]["out"]  # (128, 2048) fp16, (w, c) interleaved
        out[n, :, h0:h0 + HSH, :] = o.reshape(HSH, W, C).transpose(
            2, 0, 1).astype(np.float32)
    return out


# revision 13
# speedup vs baseline: 1.1237x; 1.0334x over previous
"""Per-pixel dynamic 5x5 conv (KernelConv2d) + leaky-relu, data-parallel on 8 TRN2 cores.

Sharding: core i <- (n = i//2, h-half = i%2); each core computes out[n, :, h0:h0+128, :].

v9 design (fp8e3 kernel stream, (w,c)-interleaved layout, 3-route tap fan-out):
- x rows stored [wpad, c]-interleaved: every dx window is a contiguous,
  16B-aligned 2048-elem read -> all DVE products run in 2x_1P at ~1135ns,
  no odd-alignment copy needed.
- kernel tensor quantized host-side to TRN fp8 E3M4 (4 mantissa bits;
  max|k|=5.4 < 15.5; measured end-to-end rel err 0.0155 < 2e-2) halving its
  HBM footprint. Taps reach fp16 product planes via 3 routes chosen to
  balance the SBUF AXI write fabric, DVE, Pool and ACT:
    CAST taps (16): SWDGE cast-DMA fp8->fp16 (gpsimd ring), DVE multiplies.
    ACT taps  (9): loaded raw fp8 (HWDGE, issue gated behind the first cast
                   DMAs), ScalarE upconverts via its engine port (saves
                   AXI-write fabric bytes), DVE multiplies.
    (GpSimd tensor ops are NOT used for products: Pool TT contends with DVE
    TT on the shared SBUF port pair and slows both ~4x.)
- PE accumulates all 26 planes as shifted-identity matmuls in fp32 PSUM
  (ldweights reused within each dy run); shift matrices ride a small DRAM
  const DMA instead of being built on-chip.
- Tail: per 512-col chunk, stop-matmul -> ACT copies PSUM->SBUF fp16 ->
  DVE scalar_tensor_tensor max(0.2*x, x) -> output DMA on the sync ring
  (input rides scalar + gpsimd rings, so sync's ring is free at the end).
- Bottom-boundary rows (x rows 128..131) via the 50-partition tail product
  + scatter matmul at queue position 13, as in v8.
"""

import os
from contextlib import ExitStack

import numpy as np
import ml_dtypes

import concourse.bass as bass
import concourse.mybir as mybir
from concourse.bass_utils import run_bass_kernel_spmd

N, C, H, W = 4, 8, 256, 256
K = 5
PAD = 2
NCORES = 8
HSH = H // 2              # 128 output rows per core
XW = (W + 4) * C          # 2080 stored x row width ((w,c) interleaved)
CD = mybir.dt.float16
F8 = mybir.dt.float8e3
NEG = 0.2
NB = 8                    # DVE product ring buffers
NWARM = 8                 # PE warm-up dummy matmuls (HAM clock-gate)
NTAP = K * K              # 25
COMBOS = [(dy, p) for dy in (1, 2, 3, 4) for p in range(HSH - dy, HSH)]
NTAIL = len(COMBOS) * K   # 50
TAILPOS = 13              # queue position of the tail product
CW = C * W                # 2048
FD = CW                   # product plane free dim

POOL_TAPS = []                         # Pool TT contends with DVE TT: unused
ACT_TAPS = [8, 10, 12, 14, 16, 18, 20, 22, 24]
ACT_DMAS = [ACT_TAPS[0:3], ACT_TAPS[3:]]   # k8 arrives in two pieces
CAST_TAPS = [p for p in range(NTAP) if p not in POOL_TAPS and p not in ACT_TAPS]
# cast DMA groups: four singletons for a fast head, then pairs
CAST_DMAS = [[p] for p in CAST_TAPS[:4]] + [
    CAST_TAPS[i:i + 2] for i in range(4, len(CAST_TAPS), 2)]
NKC = len(CAST_DMAS)
KTSLOT = {p: i for i, p in enumerate(CAST_TAPS)}
for i, p in enumerate(ACT_TAPS):
    KTSLOT[p] = len(CAST_TAPS) + i
K8SLOT = {p: i for i, p in enumerate(ACT_TAPS + POOL_TAPS)}
NKT = len(CAST_TAPS) + len(ACT_TAPS)   # 25 fp16 tap slots
NK8 = len(ACT_TAPS) + len(POOL_TAPS)   # 9 raw fp8 tap slots

_NC_CACHE = {}


def _build_nc():
    nc = bass.Bass("TRN2", target_bir_lowering=False, debug=False,
                   num_devices=NCORES)
    xe_d = nc.dram_tensor("xe", [HSH, XW], CD, kind="ExternalInput").ap()
    wt_d = nc.dram_tensor("wt", [HSH, K, HSH], CD, kind="ExternalInput").ap()
    k8_d = nc.dram_tensor("k8", [HSH, NK8, FD], F8, kind="ExternalInput").ap()
    kc_d = nc.dram_tensor("kc", [HSH, len(CAST_TAPS), FD], F8,
                          kind="ExternalInput").ap()
    tl_d = nc.dram_tensor("tl", [NTAIL, 2 * CW + HSH], CD,
                          kind="ExternalInput").ap()
    out_d = nc.dram_tensor("out", [HSH, FD], CD, kind="ExternalOutput").ap()

    with ExitStack() as ctx:
        xe = ctx.enter_context(nc.sbuf_tensor("xe_s", [HSH, XW], CD))
        wt = ctx.enter_context(nc.sbuf_tensor("wt_s", [HSH, K, HSH], CD))
        kt = ctx.enter_context(nc.sbuf_tensor("kt_s", [HSH, NKT, FD], CD))
        k8 = ctx.enter_context(nc.sbuf_tensor("k8_s", [HSH, NK8, FD], F8))
        tl = ctx.enter_context(nc.sbuf_tensor("tl_s", [NTAIL, 2 * CW + HSH], CD))
        prod = [ctx.enter_context(nc.sbuf_tensor(f"pr{b}", [HSH, FD], CD))
                for b in range(NB)]
        ptail = ctx.enter_context(nc.sbuf_tensor("ptail", [NTAIL, FD], CD))
        tmpx = ctx.enter_context(nc.sbuf_tensor("tmpx", [HSH, FD], CD))
        ot = ctx.enter_context(nc.sbuf_tensor("ot", [HSH, FD], CD))
        pt = ctx.enter_context(nc.psum_tensor("pt", [HSH, FD], mybir.dt.float32))
        scr = ctx.enter_context(nc.psum_tensor("scr", [HSH, 512], mybir.dt.float32))

        xt = tl[:, 0:CW]
        ktl = tl[:, CW:2 * CW]
        wtl = tl[:, 2 * CW:]                      # [50, 128] scatter matrix

        s_xe = ctx.enter_context(nc.semaphore("s_xe"))
        s_w = ctx.enter_context(nc.semaphore("s_w"))
        s_k8a = ctx.enter_context(nc.semaphore("s_k8a"))
        s_k8b = ctx.enter_context(nc.semaphore("s_k8b"))
        s_tl = ctx.enter_context(nc.semaphore("s_tl"))
        s_kc = [ctx.enter_context(nc.semaphore(f"s_kc{j}")) for j in range(NKC)]
        s_ac = ctx.enter_context(nc.semaphore("s_ac"))  # ACT tap converts
        s_v = ctx.enter_context(nc.semaphore("s_v"))    # DVE queue items
        s_mm = ctx.enter_context(nc.semaphore("s_mm"))  # ring items consumed
        s_c = ctx.enter_context(nc.semaphore("s_c"))    # per-chunk stop MMs
        s_t = ctx.enter_context(nc.semaphore("s_t"))    # ACT psum->sbuf chunks
        s_e = ctx.enter_context(nc.semaphore("s_e"))    # lrelu chunks done
        s_o = ctx.enter_context(nc.semaphore("s_o"))
        block = ctx.enter_context(nc.Block())

        # queue: 25 taps in (dy, dx) order with the tail inserted at TAILPOS
        queue = []
        for p in range(NTAP):
            if len(queue) == TAILPOS:
                queue.append(('L', -1))
            queue.append(('T', p))
        # DVE-produced items in queue order (taps not on Pool, plus tail)
        dve_items = [it for it in queue if it[0] == 'L' or it[1] not in POOL_TAPS]
        sv_count = {}   # queue index -> s_v threshold when this item is ready
        n = 0
        for qi, it in enumerate(queue):
            if it[0] == 'L' or it[1] not in POOL_TAPS:
                n += 1
                sv_count[qi] = n
        ring_items = [it for it in dve_items if it[0] == 'T']  # ring-buffered
        ring_idx = {}   # tap -> ring index
        for i, it in enumerate(ring_items):
            ring_idx[it[1]] = i

        @block.sync
        def _(sync):
            sync.dma_start(xe[:], xe_d).then_inc(s_xe, 16)
            sync.dma_start(wt[:], wt_d).then_inc(s_w, 16)
            for q in range(4):
                sync.wait_ge(s_e, q + 1)
                sync.dma_start(out_d[:, 512 * q:512 * (q + 1)],
                               ot[:, 512 * q:512 * (q + 1)]).then_inc(s_o, 16)
            sync.wait_ge(s_o, 64)

        @block.scalar
        def _(scalar):
            na = len(ACT_DMAS[0])
            scalar.dma_start(k8[:, 0:na], k8_d[:, 0:na]).then_inc(s_k8a, 16)
            scalar.dma_start(k8[:, na:], k8_d[:, na:]).then_inc(s_k8b, 16)
            scalar.dma_start(tl[:], tl_d).then_inc(s_tl, 16)
            for i, p in enumerate(ACT_TAPS):
                scalar.wait_ge(s_k8a if i < na else s_k8b, 16)
                scalar.activation(kt[:, KTSLOT[p]], k8[:, K8SLOT[p]],
                                  mybir.ActivationFunctionType.Copy,
                                  bias=0.0, scale=1.0).then_inc(s_ac, 1)
            for q in range(4):
                scalar.wait_ge(s_c, q + 1)
                scalar.activation(tmpx[:, 512 * q:512 * (q + 1)],
                                  pt[:, 512 * q:512 * (q + 1)],
                                  mybir.ActivationFunctionType.Copy,
                                  bias=0.0, scale=1.0).then_inc(s_t, 1)

        @block.gpsimd
        def _(gpsimd):
            for j, taps in enumerate(CAST_DMAS):
                t0 = KTSLOT[taps[0]]
                gpsimd.dma_start(kt[:, t0:t0 + len(taps)],
                                 kc_d[:, t0:t0 + len(taps)]).then_inc(s_kc[j], 16)

        @block.vector
        def _(vector):
            vector.wait_ge(s_xe, 16)
            act_seen = 0
            kc_seen = 0
            for qi, it in enumerate(queue):
                if it[0] == 'L':
                    vector.wait_ge(s_tl, 16)
                    vector.tensor_tensor(ptail[0:NTAIL], xt[0:NTAIL],
                                         ktl[0:NTAIL],
                                         op=mybir.AluOpType.mult).then_inc(s_v, 1)
                    continue
                p = it[1]
                if p in POOL_TAPS:
                    continue
                slot = KTSLOT[p]
                if p in ACT_TAPS:
                    act_seen = ACT_TAPS.index(p) + 1
                    vector.wait_ge(s_ac, act_seen)
                else:
                    j = next(j for j, taps in enumerate(CAST_DMAS) if p in taps)
                    if j >= kc_seen:
                        kc_seen = j + 1
                        vector.wait_ge(s_kc[j], 16)
                r = ring_idx[p]
                if r >= NB and r % 4 == 0:
                    vector.wait_ge(s_mm, r - 4)
                dx = p % K
                vector.tensor_tensor(prod[r % NB][:],
                                     xe[:, C * dx:C * dx + FD],
                                     kt[:, slot],
                                     op=mybir.AluOpType.mult).then_inc(s_v, 1)
            for q in range(4):
                vector.wait_ge(s_t, q + 1)
                vector.scalar_tensor_tensor(
                    ot[:, 512 * q:512 * (q + 1)],
                    tmpx[:, 512 * q:512 * (q + 1)], NEG,
                    tmpx[:, 512 * q:512 * (q + 1)],
                    op0=mybir.AluOpType.mult,
                    op1=mybir.AluOpType.max).then_inc(s_e, 1)

        @block.tensor
        def _(tensor):
            for r in range(NWARM):
                mm = tensor.matmul(scr[:], lhsT=ot[:, 0:HSH],
                                   rhs=ot[:, 0:512], start=True, stop=True)
                if r > 0:
                    mm.ins.ldweights = False
            tensor.wait_ge(s_w, 16)
            prev_w = [-1]

            def mmul(rhs_ap, q, wid, start, stop):
                mm = tensor.matmul(pt[:, 512 * q:512 * (q + 1)],
                                   lhsT=(wtl[0:NTAIL] if wid == 5
                                         else wt[:, wid]),
                                   rhs=rhs_ap,
                                   start=start, stop=stop)
                if wid == prev_w[0]:
                    mm.ins.ldweights = False
                prev_w[0] = wid
                return mm

            for qi, it in enumerate(queue):
                first, last = qi == 0, qi == len(queue) - 1
                if it[0] == 'L':
                    tensor.wait_ge(s_v, sv_count[qi])
                    for q in range(4):
                        mm = mmul(ptail[0:NTAIL, 512 * q:512 * (q + 1)], q, 5,
                                  first, last)
                    continue
                p = it[1]
                dy = p // K
                tensor.wait_ge(s_v, sv_count[qi])
                src = prod[ring_idx[p] % NB]
                for q in range(4):
                    mm = mmul(src[:, 512 * q:512 * (q + 1)], q, dy,
                              first, last)
                    if last:
                        mm.then_inc(s_c, 1)
                if not last:
                    mm.then_inc(s_mm, 1)
    return nc


def get_nc():
    if "nc" not in _NC_CACHE:
        _NC_CACHE["nc"] = _build_nc()
    return _NC_CACHE["nc"]


def _prep_shards(x: np.ndarray, kernel: np.ndarray):
    """Host-side: pad, quantize kernel to e3m4, build (w,c)-interleaved shards."""
    f16 = np.float16
    f8 = ml_dtypes.float8_e3m4
    xp = np.pad(x, ((0, 0), (0, 0), (PAD, PAD), (PAD, PAD)),
                mode='edge').astype(f16)              # (N, C, 260, 260)
    k8full = kernel.astype(f8)                         # quantize once
    kr8 = k8full.reshape(N, C, NTAP, H, W)
    kr16 = kernel.astype(f16).reshape(N, C, NTAP, H, W)

    # shift-matrix blob (shared by all cores)
    wtb = np.zeros((HSH, K, HSH), f16)
    for dy in range(K):
        for q in range(dy, HSH):
            wtb[q, dy, q - dy] = 1.0

    in_maps = []
    for core in range(NCORES):
        n, hb = divmod(core, 2)
        h0 = hb * HSH
        blk = xp[n, :, h0:h0 + HSH + 4, :]             # (C, 132, 260)
        # x rows, (w, c) interleaved
        xeb = np.ascontiguousarray(
            blk[:, :HSH, :].transpose(1, 2, 0)).reshape(HSH, XW)

        kb8 = kr8[n, :, :, h0:h0 + HSH, :]             # (C, 25, 128, W) e3m4
        kcb = np.zeros((HSH, len(CAST_TAPS), FD), f8)
        k8b = np.zeros((HSH, NK8, FD), f8)
        for p in CAST_TAPS:
            dy = p // K
            kcb[dy:, KTSLOT[p]] = kb8[:, p, :HSH - dy].transpose(
                1, 2, 0).reshape(HSH - dy, FD)
        for p in ACT_TAPS + POOL_TAPS:
            dy = p // K
            k8b[dy:, K8SLOT[p]] = kb8[:, p, :HSH - dy].transpose(
                1, 2, 0).reshape(HSH - dy, FD)

        kb16 = kr16[n, :, :, h0:h0 + HSH, :]           # (C, 25, 128, W) fp16
        tlb = np.zeros((NTAIL, 2 * CW + HSH), f16)
        for j, (dy, p) in enumerate(COMBOS):
            for dx in range(K):
                tlb[j * K + dx, 0:CW] = blk[:, p + dy, dx:dx + W].T.reshape(CW)
                tlb[j * K + dx, CW:2 * CW] = kb16[:, dy * K + dx, p].T.reshape(CW)
                tlb[j * K + dx, 2 * CW + p] = 1.0

        in_maps.append({"xe": xeb, "wt": wtb, "k8": k8b, "kc": kcb, "tl": tlb})
    return in_maps


def kernel(x: np.ndarray, kernel: np.ndarray) -> np.ndarray:
    nc = get_nc()
    in_maps = _prep_shards(np.asarray(x), np.asarray(kernel))
    trace = bool(int(os.environ.get("KC_TRACE", "0")))
    res = run_bass_kernel_spmd(nc, in_maps, core_ids=list(range(NCORES)),
                               trace=trace)
    _NC_CACHE["last_results"] = res
    out = np.empty((N, C, H, W), np.float32)
    for core in range(NCORES):
        n, hb = divmod(core, 2)
        h0 = hb * HSH
        o = res.results[core]["out"]  # (128, 2048) fp16, (w, c) interleaved
        out[n, :, h0:h0 + HSH, :] = o.reshape(HSH, W, C).transpose(
            2, 0, 1).astype(np.float32)
    return out
